# revision 6
# baseline (speedup 1.0000x reference)
"""Raw-bass TP-8 CPLSTM kernel builder.

One SPMD program for 8 cores. Core c owns:
  - H-slice Hc = [128c, 128c+128) (cell state, mm2 output)
  - rank slot c = (gate c>>1, half c&1) of the reordered 4R dim (order f,i,o,g~)

Layouts are feature-major ("transposed"): every per-step tensor is (128 feat, 32 batch).
Per step: mm1-B (a_c stationary, hT_all moving) -> DVE mul xb -> broadcast gT ->
mm2-B (ct_c stationary, gT_all moving) -> ACT sigmoid/tanh -> DVE cell -> broadcast hT.
"""
import sys

sys.path.insert(0, "/opt/trn_rl_repo")

import numpy as np
import concourse.bass as bass
import concourse.bacc as bacc
import concourse.mybir as mybir
from concourse.bass import ds
from concourse.masks import make_identity
from concourse import library_config

F32 = mybir.dt.float32
AF = mybir.ActivationFunctionType
ALU = mybir.AluOpType

B = 32
NK = 8           # k-tiles (1024/128)
KSLOT = 4        # comm buffer slots


def build(S=512, use_barrier=True, wdt="f32", h0=0.0, c0=0.0):
    """Returns nc. Inputs: x(32,S,1024) f32, a_c(8,128,128), b_c(8,128,128),
    ct_c(8,128,128). Outputs: out_hT(128,S,32), out_cT(128,32).
    wdt: dtype of a/ct weights and the h/g exchange ("f32" or "bf16")."""
    assert S % 128 == 0
    WD = F32 if wdt == "f32" else mybir.dt.bfloat16
    WB = 4 if wdt == "f32" else 2
    RCH = S // 128           # x row-chunks per batch element
    FLUSH = 64 if S >= 128 else S
    OUT_DEPTH = 2 * FLUSH
    assert S % FLUSH == 0
    BASE = 2 if use_barrier else 1   # h-bcast rounds before step 0 (barrier? + init)

    nc = bacc.Bacc("TRN2", num_devices=8, num_swdge_queues=2,
                   detect_race_conditions=False)

    # ---- DRAM I/O ----
    x_d = nc.dram_tensor("x", [B, S, 1024], F32, kind="ExternalInput")
    a_d = nc.dram_tensor("a_c", [NK, 128, 128], WD, kind="ExternalInput")
    b_d = nc.dram_tensor("b_c", [NK, 128, 128], F32, kind="ExternalInput")
    ct_d = nc.dram_tensor("ct_c", [NK, 128, 128], WD, kind="ExternalInput")
    out_d = nc.dram_tensor("out_hT", [128, S, B], F32, kind="ExternalOutput")
    outc_d = nc.dram_tensor("out_cT", [128, B], F32, kind="ExternalOutput")

    # ---- SBUF ----
    a_sb = nc.alloc_sbuf_tensor("a_sb", [128, NK, 128], WD)
    b_sb = nc.alloc_sbuf_tensor("b_sb", [128, NK, 128], F32)
    ct_sb = nc.alloc_sbuf_tensor("ct_sb", [128, NK, 128], WD)
    xbT = nc.alloc_sbuf_tensor("xbT", [128, S, B], F32)
    outst = nc.alloc_sbuf_tensor("outst", [128, OUT_DEPTH, B], F32)
    hT_all = nc.alloc_sbuf_tensor("hT_all", [128, KSLOT, 8, B], WD)
    gT_all = nc.alloc_sbuf_tensor("gT_all", [128, KSLOT, 8, B], WD)
    gT_st = nc.alloc_sbuf_tensor("gT_st", [128, KSLOT, B], WD)
    hT_st = nc.alloc_sbuf_tensor("hT_st", [128, KSLOT, B], WD)
    zeros = nc.alloc_sbuf_tensor("zeros", [128, B], F32)
    zerosw = nc.alloc_sbuf_tensor("zerosw", [128, B], WD)
    sig = nc.alloc_sbuf_tensor("sig", [128, 3 * B], F32)
    gth = nc.alloc_sbuf_tensor("gth", [128, B], F32)
    tnh = nc.alloc_sbuf_tensor("tnh", [128, B], F32)
    tp1 = nc.alloc_sbuf_tensor("tp1", [128, B], F32)
    tp2 = nc.alloc_sbuf_tensor("tp2", [128, B], F32)
    cT = nc.alloc_sbuf_tensor("cT", [128, B], F32)
    x_nat = nc.alloc_sbuf_tensor("x_nat", [128, 2, RCH, 1024], F32)
    xT_sb = nc.alloc_sbuf_tensor("xT_sb", [128, NK, S], F32)
    ident = nc.alloc_sbuf_tensor("ident", [128, 128], F32)

    # ---- PSUM ----
    ps_tr = [nc.alloc_psum_tensor(f"ps_tr{i}", [128, 512], F32) for i in range(2)]
    ps_xb = nc.alloc_psum_tensor("ps_xb", [128, 512], F32)
    ps_g = [nc.alloc_psum_tensor(f"ps_g{i}", [128, 512], F32) for i in range(2)]
    ps_y = [nc.alloc_psum_tensor(f"ps_y{i}", [128, 512], F32) for i in range(2)]

    # ---- semaphores ----
    s = {n: nc.alloc_semaphore(n) for n in
         ["wdma", "xdma", "tr", "xtc", "gemm", "ph1", "bar",
          "harr", "garr", "hloc", "gloc", "prep",
          "mm1", "mm2", "mm2a", "dvemul", "act1", "dvecell", "tnh", "dveh",
          "dvehf", "flush"]}

    RD = [(0, k) for k in range(8)]

    with nc.Block() as block:

        # ---------------- GPSIMD: init, barrier, broadcasts ----------------
        @block.gpsimd
        def _(gp):
            make_identity(nc, ident.ap())
            gp.memset(zeros.ap(), 0.0)
            gp.memset(zerosw.ap(), h0)
            gp.memset(cT.ap(), c0)
            pid = gp.partition_id()
            npre = 0
            # sem-only barrier: every core signals every core
            if use_barrier:
                gp.remote_sem_update_broadcast(
                    s["bar"], s["hloc"], rdests=RD, queue_num=1
                ).then_inc(s["prep"], 1)
                npre += 1
                gp.wait_ge(s["prep"], npre)
                gp.trigger_dma(1, queue_num=1)
                gp.wait_ge(s["bar"], 16)
            # initial zero-h broadcast (slot KSLOT-1)
            gp.remote_dma_broadcast(
                hT_all[:, KSLOT - 1, ds(pid, 1), :].opt(), zerosw[:, :],
                s["harr"], s["hloc"], rdests=RD, queue_num=1,
            ).then_inc(s["prep"], 1)
            npre += 1
            gp.wait_ge(s["prep"], npre)
            gp.trigger_dma(1, queue_num=1)

            # descriptor-gen is ~1us of Pool time per broadcast: prep AHEAD
            # steps early and issue each prep in the Pool idle window after
            # the corresponding trigger, so triggers fire sem-ready.
            AHEAD = 4
            prep_done = {}

            def prep_g(tt):
                nonlocal npre
                gp.remote_dma_broadcast(
                    gT_all[:, tt % KSLOT, ds(pid, 1), :].opt(),
                    gT_st[:, tt % KSLOT, :],
                    s["garr"], s["gloc"], rdests=RD, queue_num=0,
                ).then_inc(s["prep"], 1)
                npre += 1
                prep_done[("g", tt)] = npre

            def prep_h(tt):
                nonlocal npre
                gp.remote_dma_broadcast(
                    hT_all[:, tt % KSLOT, ds(pid, 1), :].opt(),
                    hT_st[:, tt % KSLOT, :],
                    s["harr"], s["hloc"], rdests=RD, queue_num=1,
                ).then_inc(s["prep"], 1)
                npre += 1
                prep_done[("h", tt)] = npre

            for tt in range(min(AHEAD, S)):
                prep_g(tt)
                prep_h(tt)
            for t in range(S):
                gp.wait_ge(s["prep"], prep_done[("g", t)])
                gp.wait_ge(s["dvemul"], t + 1)
                gp.trigger_dma(1, queue_num=0)
                if t + AHEAD < S:
                    prep_h(t + AHEAD)
                gp.wait_ge(s["prep"], prep_done[("h", t)])
                gp.wait_ge(s["dveh"], t + 1)
                gp.trigger_dma(1, queue_num=1)
                if t + AHEAD < S:
                    prep_g(t + AHEAD)

        # ---------------- SYNC: DMAs ----------------
        @block.sync
        def _(sy):
            sy.dma_start(a_sb.ap(), a_d.ap().rearrange("k p r -> p k r")).then_inc(s["wdma"], 16)
            sy.dma_start(b_sb.ap(), b_d.ap().rearrange("k p r -> p k r")).then_inc(s["wdma"], 16)
            sy.dma_start(ct_sb.ap(), ct_d.ap().rearrange("k p r -> p k r")).then_inc(s["wdma"], 16)
            for b in range(B):
                if b >= 2:
                    sy.wait_ge(s["tr"], NK * (b - 1))
                sy.dma_start(
                    x_nat[:, b % 2, :, :],
                    x_d[b].rearrange("(r p) i -> p r i", p=128),
                ).then_inc(s["xdma"], 16)
            for j in range(S // FLUSH):
                sy.wait_ge(s["dvehf"], FLUSH * (j + 1))
                sy.dma_start(
                    out_d[:, FLUSH * j:FLUSH * (j + 1), :],
                    outst[:, FLUSH * (j % 2):FLUSH * (j % 2) + FLUSH, :],
                ).then_inc(s["flush"], 16)
            sy.wait_ge(s["dvecell"], S)
            sy.dma_start(outc_d.ap(), cT.ap()).then_inc(s["flush"], 16)

        # ---------------- PE ----------------
        @block.tensor
        def _(te):
            te.wait_ge(s["wdma"], 48)
            # phase 1: transpose x[b] then GEMM into xbT column b
            for b in range(B):
                te.wait_ge(s["xdma"], 16 * (b + 1))
                for k in range(NK):
                    if NK * b + k >= 2:
                        te.wait_ge(s["xtc"], NK * b + k - 1)
                    mm = None
                    for r in range(RCH):
                        mm = te.transpose(
                            ps_tr[k % 2][:, 128 * r:128 * (r + 1)],
                            x_nat[:, b % 2, r, 128 * k:128 * (k + 1)],
                            ident.ap(),
                        )
                    mm.then_inc(s["tr"], 1)
                if b >= 1:
                    te.wait_ge(s["ph1"], b)
                for k in range(NK):
                    te.wait_ge(s["xtc"], NK * b + k + 1)
                    mm = te.matmul(ps_xb[:, 0:S], b_sb[:, k, :], xT_sb[:, k, :],
                                   start=(k == 0), stop=(k == NK - 1))
                mm.then_inc(s["gemm"], 1)
            # scan
            te.wait_ge(s["ph1"], B)
            for t in range(S):
                st, sh = t % KSLOT, (t - 1) % KSLOT
                pg, py = ps_g[t % 2], ps_y[t % 2]
                te.wait_ge(s["harr"], 16 * (t + 1))
                if t >= 2:
                    te.wait_ge(s["dvemul"], t - 1)
                mm = None
                for i in range(NK):
                    mm = te.matmul(pg[:, 0:B], a_sb[:, i, :], hT_all[:, sh, i, :],
                                   start=(i == 0), stop=(i == NK - 1))
                mm.then_inc(s["mm1"], 1)
                te.wait_ge(s["garr"], 16 * (t + 1))
                if t >= 2:
                    te.wait_ge(s["act1"], t - 1)
                for sl in range(8):
                    g, half = sl >> 1, sl & 1
                    # g~ gate (g==3) goes to the phase-1 transpose bank (free
                    # during scan) so sigmoid can read py while PE writes g~.
                    dst = py[:, B * g:B * (g + 1)] if g < 3 else ps_tr[t % 2][:, 0:B]
                    mm = te.matmul(dst, ct_sb[:, sl, :],
                                   gT_all[:, st, sl, :],
                                   start=(half == 0), stop=(half == 1))
                    if sl == 5:
                        mm.then_inc(s["mm2a"], 1)
                mm.then_inc(s["mm2"], 1)

        # ---------------- DVE ----------------
        @block.vector
        def _(ve):
            for b in range(B):
                for k in range(NK):
                    ve.wait_ge(s["tr"], NK * b + k + 1)
                    ve.tensor_copy(xT_sb[:, k, :], ps_tr[k % 2][:, 0:S]).then_inc(s["xtc"], 1)
                ve.wait_ge(s["gemm"], b + 1)
                ve.tensor_copy(xbT[:, :, b], ps_xb[:, 0:S]).then_inc(s["ph1"], 1)
            for t in range(S):
                st = t % KSLOT
                ve.wait_ge(s["mm1"], t + 1)
                if t >= KSLOT:
                    ve.wait_ge(s["gloc"], 16 * (t - KSLOT + 1))
                ve.tensor_tensor(gT_st[:, st, :], ps_g[t % 2][:, 0:B], xbT[:, t, :],
                                 ALU.mult).then_inc(s["dvemul"], 1)
                ve.wait_ge(s["act1"], t + 1)
                ve.tensor_tensor(tp1.ap(), sig[:, B:2 * B], gth.ap(), ALU.mult)
                ve.tensor_tensor(tp2.ap(), sig[:, 0:B], cT.ap(), ALU.mult)
                ve.tensor_tensor(cT.ap(), tp1.ap(), tp2.ap(), ALU.add).then_inc(s["dvecell"], 1)
                ve.wait_ge(s["tnh"], t + 1)
                if t >= KSLOT:
                    # h-bcast of step t-KSLOT fully sent (bf/staging slot free)
                    ve.wait_ge(s["hloc"], 16 * (t - KSLOT + BASE + 1))
                ve.tensor_tensor(hT_st[:, t % KSLOT, :], sig[:, 2 * B:3 * B],
                                 tnh.ap(), ALU.mult).then_inc(s["dveh"], 1)
                if t >= OUT_DEPTH and t % FLUSH == 0:
                    ve.wait_ge(s["flush"], 16 * (t // FLUSH - 1))
                ve.tensor_tensor(outst[:, t % OUT_DEPTH, :], sig[:, 2 * B:3 * B],
                                 tnh.ap(), ALU.mult).then_inc(s["dvehf"], 1)

        # ---------------- ACT ----------------
        @block.scalar
        def _(sc):
            bias0 = zeros[:, 0:1]
            for t in range(S):
                sc.wait_ge(s["mm2a"], t + 1)
                sc.activation(sig.ap(), ps_y[t % 2][:, 0:3 * B], AF.Sigmoid, bias=bias0)
                sc.wait_ge(s["mm2"], t + 1)
                sc.activation(gth.ap(), ps_tr[t % 2][:, 0:B], AF.Tanh,
                              bias=bias0).then_inc(s["act1"], 1)
                sc.wait_ge(s["dvecell"], t + 1)
                sc.activation(tnh.ap(), cT.ap(), AF.Tanh, bias=bias0).then_inc(s["tnh"], 1)

    nc.compile()
    return nc


# ---------------- host-side shard prep (mirrors golden.py) ----------------
GATE_PERM = [0, 1, 3, 2]


def reorder_gates(m, axis):
    blocks = np.split(np.asarray(m), 4, axis=axis)
    return np.concatenate([blocks[g] for g in GATE_PERM], axis=axis)


def make_in_maps(x, a, b, ct, wdt="f32"):
    import ml_dtypes
    wnp = np.float32 if wdt == "f32" else ml_dtypes.bfloat16
    a2 = reorder_gates(a, 1).astype(np.float32)
    b2 = reorder_gates(b, 1).astype(np.float32)
    ct2 = reorder_gates(ct, 0).astype(np.float32)
    x = np.ascontiguousarray(np.asarray(x, np.float32))
    maps = []
    for c in range(8):
        rr = slice(128 * c, 128 * (c + 1))
        hc = slice(128 * c, 128 * (c + 1))
        a_c = np.stack([a2[128 * k:128 * (k + 1), rr] for k in range(8)])
        b_c = np.stack([b2[128 * k:128 * (k + 1), rr] for k in range(8)])
        ct_c = np.stack([ct2[128 * sl:128 * (sl + 1), hc] for sl in range(8)])
        maps.append({"x": x, "a_c": np.ascontiguousarray(a_c).astype(wnp),
                     "b_c": np.ascontiguousarray(b_c),
                     "ct_c": np.ascontiguousarray(ct_c).astype(wnp)})
    return maps


def unshard(results, S=512):
    """results: list of 8 dicts with out_hT (128,S,32), out_cT (128,32)."""
    hs = np.stack([r["out_hT"] for r in results])      # (8,128,S,32)
    hs = hs.transpose(3, 2, 0, 1).reshape(B, S, 1024)  # b,t,(c,hc)
    c_fin = np.stack([r["out_cT"] for r in results]).transpose(2, 0, 1).reshape(B, 1024)
    return hs, (hs[:, -1, :].copy(), c_fin)


# ---------------- entry point ----------------
_CACHED_NC = {}
LAST_RESULT = None  # BassKernelResults of the most recent run (for profiling)
TRACE = False
WDT = "bf16"  # weight/exchange dtype: "bf16" (fast) or "f32" (bit-exact)


def _get_nc(wdt):
    if wdt not in _CACHED_NC:
        _CACHED_NC[wdt] = build(S=512, wdt=wdt)
    return _CACHED_NC[wdt]


def kernel(x, a, b, ct):
    global LAST_RESULT
    from concourse.bass_utils import run_bass_kernel_spmd

    nc = _get_nc(WDT)
    in_maps = make_in_maps(x, a, b, ct, wdt=WDT)
    res = run_bass_kernel_spmd(nc, in_maps, core_ids=list(range(8)), trace=TRACE)
    LAST_RESULT = res
    return unshard(res.results, S=512)
